# revision 1
# baseline (speedup 1.0000x reference)
"""Trainium2 Bass kernel for the real-space Ewald sum (nn_Ewald).

Math (per molecule b, nb=2048 atoms, 8 charge channels):
    pot_b = sum_{i,j} qq_ij * erf(|rij|/sqrt(2)) / (|rij|+1e-6) / (4*pi)
          + sum_i qq_ii / (2*pi)^1.5            (self term)
    all scaled by NORM_FACTOR.

Key identity: w(s) = erf(sqrt(s/2))/sqrt(s) (s = |rij|^2) is analytic in s
near 0 and equals 1/sqrt(s) to <1e-3 for s > ~9.  So per pair tile:
    s'    = delta * s                 (K=13 bf16 hi/lo augmented matmul;
                                       delta = 2^-3 keeps products exact)
    y     = kappa/sqrt(s)             (one ACT pass: Abs_reciprocal_sqrt,
                                       scale = 1/(delta*kappa^2))
    w'    = min(y, max(P(s'), 1))     (ONE custom DVE op; P = monic cubic
                                       ((C0-s')s'+C1)s'+C2 fit to
                                       kappa*w(s) on s in [0, 9])
    u[c,j] += sum_i q_bf16[i,c] w'_ij (bf16 PE contraction, f32 PSUM)
Host: pot_b = (sum(u * q^T) - sum_i qq_ii*w'_ii)/kappa/(4*pi) + self term.
The diagonal is NOT masked on device; its value w'_ii = P(eps_ii) is
replicated on the host (eps_ii = the deterministic hi/lo rounding residue
of s'_ii) and subtracted exactly.  No erf table, no second ACT pass, no
second DVE pass, fully streaming (no phase barrier).

Symmetry: w is symmetric, so only the block upper triangle is computed.
Row block rb (0..15 within molecule, 128 atoms) covers col windows
jc >= rb//4; the diagonal 512-block gets weight 1 (both orientations of
in-group pairs are computed), strictly-above blocks get weight 2. Each
core takes 8 row blocks whose triangle widths form the multiset
{4,4,3,3,2,2,1,1} so all 8 cores run the identical program (SPMD).

Sharding: 8 cores = 4 molecules x 2 row-block sets.
"""

import numpy as np

B = 4
NB = 2048
NQ = 8
NCORES = 8
RB = 8              # row blocks per core
CT = 512            # matmul col tile (PSUM bank)
NORM_FACTOR = 90.0474
KA = 13             # augmented contraction depth (bf16 hi/lo split)

# fitted scheme constants (see fit in repo history): s_c=9, delta=2^-3
DELTA = 0.125
KAPPA = 3.0708577931200534
PC0 = 2.7432632222505378      # s0 -> C0
PC1 = -3.112066562880879      # s1 -> C1
PC2 = 2.4529603188601343      # imm2 -> C2
ACT_SCALE = 1.0 / (DELTA * KAPPA * KAPPA)

# per-slot triangle width in 512-blocks; identical multiset on every core
NJC = [4, 4, 3, 3, 2, 2, 1, 1]
# row blocks (within molecule) per core half
SLOT_RBG = {
    0: [0, 1, 4, 5, 8, 9, 12, 13],
    1: [2, 3, 6, 7, 10, 11, 14, 15],
}
# chunk widths per slot (PSUM tile granularity: 512 = 1 bank, 4-deep
# rotation keeps the A->ACT->DVE->A slot-recycle loop off the critical path)
CHUNKS = {4: [512] * 4, 3: [512] * 3, 2: [512] * 2, 1: [512]}
# last slot contributing to each u bank (first is always slot 0)
BANK_LAST_SLOT = {0: 1, 1: 3, 2: 5, 3: 7}

_compiled = None
_ops = None


def _register_ops():
    """Register the fused Ewald-weight DVE op (idempotent)."""
    global _ops
    if _ops is not None:
        return _ops
    from concourse import dve_ops
    from concourse.dve_spec import (
        Spec, Src0, Src1, C0, C1, C2, One, lower, _has_src1, minn, maxx,
    )
    from concourse.dve_uop import DveOpSpec

    def mk(name, spec):
        for o in dve_ops.OPS:
            if o.name == name:
                return o
        shas = {}
        for ver in ("v3", "v4"):
            tmp = DveOpSpec(
                name=name,
                opcode=31,
                uops=lower(spec, ver=ver),
                rd1_en=_has_src1(spec),
            )
            shas[ver] = tmp.sha(ver)
        op = dve_ops.DveOp(name, spec, subdim=False, uops_sha=shas)
        dve_ops.OPS.append(op)
        dve_ops._SUB_OPCODE_FOR_NAME[name] = (
            dve_ops._CUSTOM_DVE_ROW_BASE + len(dve_ops.OPS) - 1
        )
        dve_ops.CUSTOM_DVE_SPECS[name] = spec
        return op

    # w' = min(y, max(((C0 - s')s' + C1)s' + C2, 1))
    def _pw_ref(in0, in1, s0, s1, imm2):
        sp = in0.astype(np.float32)
        y = in1.astype(np.float32)
        p = ((np.float32(s0) - sp) * sp + np.float32(s1)) * sp + np.float32(imm2)
        return np.minimum(y, np.maximum(p, np.float32(1.0))).astype(np.float32)

    ewald_pw = mk(
        "EWALD_PW",
        Spec(
            body=minn(Src1, maxx(((C0 - Src0) * Src0 + C1) * Src0 + C2, One)),
            reference=_pw_ref,
        ),
    )
    _ops = (ewald_pw,)
    return _ops


def build_nc():
    """Build + compile the per-core Bass program (fixed shapes)."""
    from concourse import bacc, tile
    import concourse.mybir as mybir
    from concourse.bass import ts, ds
    from concourse.tile_rust import add_dep_helper

    (ewald_pw,) = _register_ops()
    f32 = mybir.dt.float32
    bf16 = mybir.dt.bfloat16
    AF = mybir.ActivationFunctionType

    nc = bacc.Bacc(
        "TRN2",
        target_bir_lowering=False,
        debug=False,
        num_devices=NCORES,
    )
    # aug split: slot-ordered stationary block and the full col block
    augs = nc.dram_tensor("augs", [KA, RB * 128], bf16, kind="ExternalInput").ap()
    augm = nc.dram_tensor("augm", [KA, NB], bf16, kind="ExternalInput").ap()
    # q12 = [q1 | q2] stacked on the free dim (one DMA)
    q12 = nc.dram_tensor(
        "q12", [128, 2 * RB * NQ], bf16, kind="ExternalInput"
    ).ap()
    uout = nc.dram_tensor("uout", [NQ, NB], f32, kind="ExternalOutput").ap()

    with tile.TileContext(nc) as tc:
        with (
            tc.tile_pool(name="const", bufs=1) as cpool,
            tc.tile_pool(name="work", bufs=4) as wpool,
            tc.tile_pool(name="ps", bufs=1, space="PSUM") as ps,
        ):
            # split input DMAs; first chunk's operands issued in parallel
            # on different queues so they land early.
            augm_sb = cpool.tile([KA, NB], bf16)
            nc.sync.dma_start(out=augm_sb[:, 0:512], in_=augm[:, 0:512])
            augs_sb = cpool.tile([KA, RB * 128], bf16)
            nc.scalar.dma_start(out=augs_sb[:, 0:128], in_=augs[:, 0:128])
            nc.sync.dma_start(out=augm_sb[:, 512:NB], in_=augm[:, 512:NB])
            nc.scalar.dma_start(
                out=augs_sb[:, 128 : RB * 128], in_=augs[:, 128 : RB * 128]
            )
            q12_sb = cpool.tile([128, 2 * RB * NQ], bf16)
            nc.gpsimd.dma_start(out=q12_sb[:], in_=q12[:])
            q1_sb = q12_sb[:, 0 : RB * NQ]
            q2_sb = q12_sb[:, RB * NQ : 2 * RB * NQ]

            # PSUM: 4 x [128,512] s tiles (4 banks) + 4 x [8,512] u banks.
            # u banks 0/1 retire early (slots 1/3) and their banks are
            # recycled as extra s slots, deepening the pipeline.
            u_tiles = [
                ps.tile([NQ, CT], f32, tag=f"u{j}", bufs=1, name=f"u_ps{j}")
                for j in range(4)
            ]
            u_stage = {}

            # flat per-chunk y/w regions: no buffer rotation -> no slot
            # reuse deps, fewer rendezvous conditions per chunk
            NCH = sum(NJC)
            y_all = cpool.tile([128, NCH * CT], f32)
            w_all = cpool.tile([128, NCH * CT], bf16)

            # flat chunk list: (slot t, jc, col) with cw == CT always
            chunk_list = []
            for t in range(RB):
                col0 = NB - NJC[t] * CT  # window start col in molecule
                jc0 = 4 - NJC[t]         # first (diagonal) 512-block
                for ci in range(NJC[t]):
                    chunk_list.append((t, jc0 + ci, col0 + ci * CT, jc0))
            n_chunks = len(chunk_list)

            prev_mm = [None]

            def pin(inst):
                # keep the tensor queue in emission order so same-stationary
                # matmuls stay adjacent (their LDWEIGHTS swaps are hidden)
                if prev_mm[0] is not None:
                    add_dep_helper(
                        inst.ins, prev_mm[0].ins, sync=False,
                        reason="tensor queue order: batch same-stationary",
                    )
                prev_mm[0] = inst

            def emit_b(k, w_tile):
                """Phase-B matmul for chunk k."""
                t, jc, _, jc0 = chunk_list[k]
                qs = q1_sb if jc == jc0 else q2_sb
                pin(nc.tensor.matmul(
                    u_tiles[jc][:],
                    qs[:, ds(t * NQ, NQ)],
                    w_tile,
                    start=(t == 0),
                    stop=(t == BANK_LAST_SLOT[jc]),
                ))

            def drain_u(t):
                """Copy + DMA any u bank whose accumulation ended at slot t."""
                for jb in range(4):
                    if BANK_LAST_SLOT[jb] == t:
                        pair = jb // 2
                        if jb % 2 == 0:
                            u_stage[pair] = wpool.tile(
                                [NQ, 2 * CT], f32, tag="u_sb", bufs=2,
                                name="u_sb",
                            )
                        nc.scalar.copy(
                            u_stage[pair][:, ts(jb % 2, CT)],
                            u_tiles[jb][:],
                        )
                        if jb % 2 == 1:
                            nc.gpsimd.dma_start(
                                out=uout[:, ds(pair * 2 * CT, 2 * CT)],
                                in_=u_stage[pair][:],
                            )

            # Slot-batched emission: all A-matmuls of slot t share one
            # stationary (augs[t]) and run back-to-back (same-weight
            # LDWEIGHTS are free); then the lagging B-group of slot t-1
            # runs q1-chunk first, q2-chunks contiguously — 3 weight
            # swaps per slot instead of 2 per chunk.
            w_tiles = {}
            slot_chunks = {}
            for k, (t, jc, col, jc0) in enumerate(chunk_list):
                slot_chunks.setdefault(t, []).append(k)

            def emit_b_lagged(k):
                emit_b(k, w_tiles.pop(k))
                t = chunk_list[k][0]
                if k == slot_chunks[t][-1]:
                    drain_u(t)

            # chunks that reuse a retired u bank as a 5th/6th s slot
            # (u0 retires after B(7) is emitted, u1 after B(13); leave
            # slack after the copy so the reuse never serializes)
            RECYCLE = {}
            LAG = 4  # B trails A by LAG chunks, emitted in pairs
            for k, (t, jc, col, jc0) in enumerate(chunk_list):
                tag, nb = RECYCLE.get(k, ("s", 4))
                s_ps = ps.tile([128, CT], f32, tag=tag, bufs=nb,
                               name="s_ps")
                pin(nc.tensor.matmul(
                    s_ps[:],
                    augs_sb[:, ts(t, 128)],
                    augm_sb[:, ds(col, CT)],
                    start=True,
                    stop=True,
                ))
                y_sb = y_all[:, ds(k * CT, CT)]
                nc.scalar.activation(
                    y_sb, s_ps[:], AF.Abs_reciprocal_sqrt,
                    scale=ACT_SCALE,
                )
                w_sb = w_all[:, ds(k * CT, CT)]
                nc.vector._custom_dve(
                    ewald_pw,
                    out=w_sb,
                    in0=s_ps[:],
                    in1=y_sb,
                    s0=PC0,
                    s1=PC1,
                    imm2=PC2,
                )
                w_tiles[k] = w_sb
                if k >= LAG and k % 2 == 1:
                    emit_b_lagged(k - LAG - 1)
                    emit_b_lagged(k - LAG)
            for k in range(n_chunks - LAG, n_chunks):
                emit_b_lagged(k)

    nc.compile()
    return nc


def _make_mol(rm, qm):
    """Per-molecule hi/lo split (shared by in-map builder and host diag)."""
    import ml_dtypes

    bf = ml_dtypes.bfloat16
    rc = (rm - rm.mean(0, keepdims=True)).astype(np.float32)
    hi = rc.astype(bf)
    lo = (rc - hi.astype(np.float32)).astype(bf)
    rr = hi.astype(np.float32) + lo.astype(np.float32)
    n2 = (rr * rr).sum(1).astype(np.float32)
    n2_hi = n2.astype(bf)
    n2_lo = (n2 - n2_hi.astype(np.float32)).astype(bf)
    return hi, lo, n2_hi, n2_lo


def _aug_rows(hi, lo, n2_hi, n2_lo, rowsel):
    """The 13 (L, R) aug row pairs; R is scaled by DELTA (exact: 2^-3)."""
    import ml_dtypes

    bf = ml_dtypes.bfloat16
    n = hi.shape[0]
    dl = np.float32(DELTA)
    ones_i = np.ones(len(rowsel), bf)
    ones_j = np.full(n, dl, np.float32).astype(bf)  # delta exact in bf16
    rowsL, rowsR = [], []
    for ax in range(3):
        m2h = (-2.0 * hi[:, ax].astype(np.float32)).astype(bf)
        m2l = (-2.0 * lo[:, ax].astype(np.float32)).astype(bf)
        m2hd = (m2h.astype(np.float32) * dl).astype(bf)
        m2ld = (m2l.astype(np.float32) * dl).astype(bf)
        rowsL += [hi[rowsel, ax], hi[rowsel, ax], lo[rowsel, ax]]
        rowsR += [m2hd, m2ld, m2hd]
    n2_hid = (n2_hi.astype(np.float32) * dl).astype(bf)
    n2_lod = (n2_lo.astype(np.float32) * dl).astype(bf)
    rowsL += [n2_hi[rowsel], n2_lo[rowsel], ones_i, ones_i]
    rowsR += [ones_j, ones_j, n2_hid, n2_lod]
    return rowsL, rowsR


def make_in_maps(q, r):
    """Host-side sharding: per-core augmented bf16 hi/lo matrices."""
    import ml_dtypes

    bf = ml_dtypes.bfloat16
    q = np.ascontiguousarray(np.asarray(q, np.float32))
    r = np.ascontiguousarray(np.asarray(r, np.float32))
    in_maps = []
    for core in range(NCORES):
        b, h = core // 2, core % 2
        rm = r[b * NB : (b + 1) * NB]
        qm = q[b * NB : (b + 1) * NB]
        hi, lo, n2_hi, n2_lo = _make_mol(rm, qm)

        rbgs = SLOT_RBG[h]
        rowsel = np.concatenate(
            [np.arange(g * 128, (g + 1) * 128) for g in rbgs]
        )
        rowsL, rowsR = _aug_rows(hi, lo, n2_hi, n2_lo, rowsel)
        augs_np = np.ascontiguousarray(np.stack(rowsL).astype(bf))
        augm_np = np.ascontiguousarray(np.stack(rowsR).astype(bf))

        qi = qm[rowsel]  # [RB*128, NQ] slot-ordered
        q1_np = (
            qi.reshape(RB, 128, NQ).transpose(1, 0, 2).reshape(128, RB * NQ)
        ).astype(bf)
        q2_np = (2.0 * q1_np.astype(np.float32)).astype(bf)
        q12_np = np.ascontiguousarray(np.concatenate([q1_np, q2_np], axis=1))

        in_maps.append(
            {
                "augs": augs_np,
                "augm": augm_np,
                "q12": q12_np,
            }
        )
    return in_maps


def _host_diag_w(rm):
    """Replicate the device's diagonal weight w'_ii (f32 k-ordered accum)."""
    hi, lo, n2_hi, n2_lo = _make_mol(rm, None)
    rowsel = np.arange(NB)
    rowsL, rowsR = _aug_rows(hi, lo, n2_hi, n2_lo, rowsel)
    eps = np.zeros(NB, np.float32)
    for L, R in zip(rowsL, rowsR):
        eps = eps + L.astype(np.float32) * R.astype(np.float32)
    eps64 = eps.astype(np.float64)
    p = ((PC0 - eps64) * eps64 + PC1) * eps64 + PC2
    with np.errstate(divide="ignore"):
        y = 1.0 / np.sqrt(np.abs(ACT_SCALE * eps64))
    return np.minimum(y, np.maximum(p, 1.0))


def reduce_outputs(q, r, results):
    """Host-side gather: u[8,2048] per core -> pot[B].

    The device computes the diagonal with weight w'_ii = P(eps_ii)
    (eps_ii = deterministic rounding residue of s'_ii); replicate it
    here and subtract, then add the self term in f64.
    """
    q = np.asarray(q, np.float32)
    r = np.asarray(r, np.float32)
    TWOPI = 2.0 * np.pi
    pots = np.zeros(B, np.float64)
    for core in range(NCORES):
        b = core // 2
        u = results[core]["uout"].astype(np.float64)
        qm = q[b * NB : (b + 1) * NB].astype(np.float64)
        pots[b] += (u * qm.T).sum()
    for b in range(B):
        rm = r[b * NB : (b + 1) * NB]
        qm = q[b * NB : (b + 1) * NB].astype(np.float64)
        wdiag = _host_diag_w(rm)
        pots[b] -= ((qm**2).sum(1) * wdiag).sum()
    pots = pots / KAPPA / (4.0 * np.pi)
    for b in range(B):
        qm = q[b * NB : (b + 1) * NB].astype(np.float64)
        pots[b] += (qm**2).sum() / ((2.0 * np.pi) ** 1.5)
    return (pots * NORM_FACTOR).astype(np.float32)


def kernel(q, r, batch):
    global _compiled
    if _compiled is None:
        _compiled = build_nc()
    from concourse import bass_utils

    in_maps = make_in_maps(q, r)
    last_err = None
    for attempt in range(3):
        try:
            res = bass_utils.run_bass_kernel_spmd(
                _compiled, in_maps, core_ids=list(range(NCORES))
            )
            return reduce_outputs(q, r, res.results)
        except Exception as e:  # transient device errors: back off and retry
            last_err = e
            import time

            time.sleep(15 * (attempt + 1))
    raise last_err



# revision 7
# speedup vs baseline: 1.0568x; 1.0568x over previous
"""Trainium2 Bass kernel for the real-space Ewald sum (nn_Ewald) — v2.

Math (per molecule, 2048 atoms, 8 charge channels):
    pot = sum_{i,j} qq_ij * erf(|rij|/sqrt(2)) / (|rij|+1e-6) / (4*pi)
        + sum_i qq_ii / (2*pi)^1.5   (self term), scaled by NORM_FACTOR.

Per pair tile the device computes (as in v1):
    s' = delta*s  (K=13 bf16 hi/lo augmented matmul, delta=2^-3 exact)
    y  = kappa/sqrt(s)        (ACT Abs_reciprocal_sqrt -> bf16)
    w' = min(y, max(P(s'),1)) (one custom DVE op, monic cubic P -> bf16)
    u[c,j] += q[i,c] w'_ij    (PE contraction, f32 PSUM)
Host: pot = (sum(u*q^T) - sum_i qq_ii w'_ii)/kappa/(4*pi) + self term.

v2 geometry — circulant tournament cover at 128-atom granularity:
  16 blocks per molecule; slot centered at block c covers the cyclic
  block window [c, c+8] (9 blocks) for c in 0..7, [c, c+7] (8 blocks)
  for c in 8..15.  Every unordered block pair is covered exactly once
  (difference 1..7 -> lower center; difference 8 -> center < 8; diag ->
  own center with weight 1 via q1, off-center blocks weight 2 via q2).
  Total 68 blocks/core = 8704 pair-columns (vs 10240 in v1, -15%).

SPMD: every core runs slots of widths {9,9,9,9,8,8,8,8} at FIXED window
positions [128j, 128j+1152) (j=0..3) and [128(8+j), 128(8+j)+1024) over
a 19-block extended column space.  Per-core data: augm_ext columns are
the molecule's blocks in cyclic order rotated by 0 (even core: centers
0-3,8-11) or 4 (odd core: centers 4-7,12-15); augs/q12 hold the 8
center blocks.  8 cores = 4 molecules x 2 center sets.

u accumulates in 2 PSUM banks (bands of 512 columns packed at partition
offsets 0/32/64/96 via matmul tile_position); banks are memset to zero
so all matmuls accumulate (start=False).  Host folds the 19-position
extended u back onto the 16 blocks.

PE runs at 1.2 GHz on this platform (HAM clock gate never releases), so
the kernel is PE-stream-bound: A 8704 + B 8704 columns ~= 14.5 us/core.
"""

import numpy as np

B = 4
NB = 2048
NQ = 8
NCORES = 8
NSLOT = 8
NPOS = 19            # extended column space, in 128-blocks
NEXT = NPOS * 128    # 2432 extended columns
NORM_FACTOR = 90.0474
KA = 13              # augmented contraction depth (bf16 hi/lo split)

# fitted scheme constants (see v1): s_c=9, delta=2^-3
DELTA = 0.125
KAPPA = 3.0708577931200534
PC0 = 2.7432632222505378
PC1 = -3.112066562880879
PC2 = 2.4529603188601343
ACT_SCALE = 1.0 / (DELTA * KAPPA * KAPPA)

# slot windows in the extended space: (start, width) in columns
SLOT_WIN = [(128 * j, 1152) for j in range(4)] + [
    (128 * (8 + j), 1024) for j in range(4)
]
# centers per core half h (h=0: rotation 0, h=1: rotation 4)
CENTERS = {0: [0, 1, 2, 3, 8, 9, 10, 11], 1: [4, 5, 6, 7, 12, 13, 14, 15]}
ROT = {0: 0, 1: 4}

U_BANDS = 5          # ceil(2432/512); 3 bands/bank at partition offsets
                     # 0/32/64 (matmul out base partition must be 0/32/64)


def _chunks():
    """Chunk list: (slot t, ext col0, width, w_all offset)."""
    out = []
    acc = 0
    for t, (ws, w) in enumerate(SLOT_WIN):
        main = min(1024, w)
        out.append((t, ws, main, acc))
        acc += main
        if w > main:
            out.append((t, ws + main, w - main, acc))
            acc += w - main
    return out


CHUNKS = _chunks()
NCOLS = sum(c[2] for c in CHUNKS)    # 8704


def _b_pieces(t, col0, cw):
    """B pieces for a chunk: (qsel, piece col0, width); splits at the
    512 band grid and at the 128-wide q1 (diag) prefix of each slot."""
    ws, w = SLOT_WIN[t]
    end = col0 + cw
    pieces = []
    p = col0
    if col0 == ws:                      # first chunk: q1 diag prefix
        pieces.append((1, p, 128))
        p += 128
    while p < end:
        nxt = min(end, (p // 512 + 1) * 512)
        pieces.append((2, p, nxt - p))
        p = nxt
    return pieces


_compiled = None
_ops = None


def _register_ops():
    """Register the fused Ewald-weight DVE op (idempotent)."""
    global _ops
    if _ops is not None:
        return _ops
    from concourse import dve_ops
    from concourse.dve_spec import (
        Spec, Src0, Src1, C0, C1, C2, One, lower, _has_src1, minn, maxx,
    )
    from concourse.dve_uop import DveOpSpec

    def mk(name, spec):
        for o in dve_ops.OPS:
            if o.name == name:
                return o
        shas = {}
        for ver in ("v3", "v4"):
            tmp = DveOpSpec(
                name=name,
                opcode=31,
                uops=lower(spec, ver=ver),
                rd1_en=_has_src1(spec),
            )
            shas[ver] = tmp.sha(ver)
        op = dve_ops.DveOp(name, spec, subdim=False, uops_sha=shas)
        dve_ops.OPS.append(op)
        dve_ops._SUB_OPCODE_FOR_NAME[name] = (
            dve_ops._CUSTOM_DVE_ROW_BASE + len(dve_ops.OPS) - 1
        )
        dve_ops.CUSTOM_DVE_SPECS[name] = spec
        return op

    # w' = min(y, max(((C0 - s')s' + C1)s' + C2, 1))
    def _pw_ref(in0, in1, s0, s1, imm2):
        sp = in0.astype(np.float32)
        y = in1.astype(np.float32)
        p = ((np.float32(s0) - sp) * sp + np.float32(s1)) * sp + np.float32(imm2)
        return np.minimum(y, np.maximum(p, np.float32(1.0))).astype(np.float32)

    ewald_pw = mk(
        "EWALD_PW",
        Spec(
            body=minn(Src1, maxx(((C0 - Src0) * Src0 + C1) * Src0 + C2, One)),
            reference=_pw_ref,
        ),
    )
    _ops = (ewald_pw,)
    return _ops


def build_nc():
    """Build + compile the per-core Bass program (fixed shapes, SPMD)."""
    from concourse import bacc, tile
    import concourse.mybir as mybir
    from concourse.bass import ds
    from concourse.tile_rust import add_dep_helper

    (ewald_pw,) = _register_ops()
    f32 = mybir.dt.float32
    bf16 = mybir.dt.bfloat16
    AF = mybir.ActivationFunctionType

    nc = bacc.Bacc(
        "TRN2",
        target_bir_lowering=False,
        debug=False,
        num_devices=NCORES,
    )
    augs = nc.dram_tensor("augs", [KA, NSLOT * 128], bf16, kind="ExternalInput").ap()
    augm = nc.dram_tensor("augm", [KA, NEXT], bf16, kind="ExternalInput").ap()
    q12 = nc.dram_tensor("q12", [128, 2 * NSLOT * NQ], bf16, kind="ExternalInput").ap()
    # raw dump of the two u PSUM banks (bands packed at partition offsets)
    uout = nc.dram_tensor("uout", [72, 1024], f32, kind="ExternalOutput").ap()

    with tile.TileContext(nc) as tc:
        with (
            tc.tile_pool(name="const", bufs=1) as cpool,
            tc.tile_pool(name="ps", bufs=1, space="PSUM") as ps,
        ):
            # ---- input DMAs (sync + gpsimd queues; scalar only table-loads)
            augm_sb = cpool.tile([KA, NEXT], bf16)
            nc.sync.dma_start(out=augm_sb[:, 0:1536], in_=augm[:, 0:1536])
            augs_sb = cpool.tile([KA, NSLOT * 128], bf16)
            nc.gpsimd.dma_start(out=augs_sb[:], in_=augs[:])
            nc.sync.dma_start(out=augm_sb[:, 1536:NEXT], in_=augm[:, 1536:NEXT])
            q12_sb = cpool.tile([128, 2 * NSLOT * NQ], bf16)
            nc.gpsimd.dma_start(out=q12_sb[:], in_=q12[:])

            # ---- u banks: bands of 512 at partition offsets, zeroed
            u_banks = [
                ps.tile([128, 512], f32, tag=f"u{i}", bufs=1, name=f"u{i}")
                for i in range(2)
            ]
            nc.vector.memset(u_banks[0][:], 0.0)
            nc.vector.memset(u_banks[1][:], 0.0)

            # flat y/w regions (no rotation -> no reuse deps)
            y_all = cpool.tile([128, NCOLS], bf16)
            w_all = cpool.tile([128, NCOLS], bf16)

            prev_mm = [None]

            def pin(inst):
                if prev_mm[0] is not None:
                    add_dep_helper(
                        inst.ins, prev_mm[0].ins, sync=False,
                        reason="tensor queue order",
                    )
                prev_mm[0] = inst

            n_chunks = len(CHUNKS)
            s_tiles = {}

            def emit_a(k):
                t, col0, cw, _ = CHUNKS[k]
                s_ps = ps.tile([128, 1024], f32, tag="s", bufs=3, name="s_ps")
                s_tiles[k] = s_ps
                off = 0
                while off < cw:
                    pw = min(512, cw - off)
                    pin(nc.tensor.matmul(
                        s_ps[:, ds(off, pw)],
                        augs_sb[:, ds(t * 128, 128)],
                        augm_sb[:, ds(col0 + off, pw)],
                        start=True,
                        stop=True,
                    ))
                    off += pw

            def emit_act_dve(k):
                t, col0, cw, woff = CHUNKS[k]
                s_ps = s_tiles[k]
                nc.scalar.activation(
                    y_all[:, ds(woff, cw)], s_ps[:, 0:cw],
                    AF.Abs_reciprocal_sqrt, scale=ACT_SCALE,
                )
                nc.vector._custom_dve(
                    ewald_pw,
                    out=w_all[:, ds(woff, cw)],
                    in0=s_ps[:, 0:cw],
                    in1=y_all[:, ds(woff, cw)],
                    s0=PC0,
                    s1=PC1,
                    imm2=PC2,
                )

            def emit_b(k):
                t, col0, cw, woff = CHUNKS[k]
                for qsel, pc0, pcw in _b_pieces(t, col0, cw):
                    band = pc0 // 512
                    bank, sub = band // 3, band % 3
                    qoff = (0 if qsel == 1 else NSLOT * NQ) + t * NQ
                    pin(nc.tensor.matmul(
                        u_banks[bank][ds(32 * sub, 8), ds(pc0 - 512 * band, pcw)],
                        q12_sb[:, ds(qoff, NQ)],
                        w_all[:, ds(woff + (pc0 - col0), pcw)],
                        start=False,
                        stop=False,
                        skip_group_check=True,
                    ))
                s_tiles.pop(k)

            LAG = 3
            for k in range(n_chunks):
                if k >= LAG:
                    emit_b(k - LAG)
                emit_a(k)
                emit_act_dve(k)
            for k in range(n_chunks - LAG, n_chunks):
                emit_b(k)

            # ---- evict u: one copy of both banks, one DMA of the used rows
            staged = cpool.tile([128, 1024], f32)
            nc.scalar.copy(staged[:, 0:512], u_banks[0][:])
            nc.scalar.copy(staged[:, 512:1024], u_banks[1][:])
            nc.sync.dma_start(out=uout[:], in_=staged[0:72, :])

    nc.compile()
    return nc


def _make_mol(rm):
    """Per-molecule hi/lo split (shared by in-map builder and host diag)."""
    import ml_dtypes

    bf = ml_dtypes.bfloat16
    rc = (rm - rm.mean(0, keepdims=True)).astype(np.float32)
    hi = rc.astype(bf)
    lo = (rc - hi.astype(np.float32)).astype(bf)
    rr = hi.astype(np.float32) + lo.astype(np.float32)
    n2 = (rr * rr).sum(1).astype(np.float32)
    n2_hi = n2.astype(bf)
    n2_lo = (n2 - n2_hi.astype(np.float32)).astype(bf)
    return hi, lo, n2_hi, n2_lo


def _aug_rows(hi, lo, n2_hi, n2_lo, rowsel):
    """The 13 (L, R) aug row pairs; R is scaled by DELTA (exact: 2^-3)."""
    import ml_dtypes

    bf = ml_dtypes.bfloat16
    n = hi.shape[0]
    dl = np.float32(DELTA)
    ones_i = np.ones(len(rowsel), bf)
    ones_j = np.full(n, dl, np.float32).astype(bf)
    rowsL, rowsR = [], []
    for ax in range(3):
        m2h = (-2.0 * hi[:, ax].astype(np.float32)).astype(bf)
        m2l = (-2.0 * lo[:, ax].astype(np.float32)).astype(bf)
        m2hd = (m2h.astype(np.float32) * dl).astype(bf)
        m2ld = (m2l.astype(np.float32) * dl).astype(bf)
        rowsL += [hi[rowsel, ax], hi[rowsel, ax], lo[rowsel, ax]]
        rowsR += [m2hd, m2ld, m2hd]
    n2_hid = (n2_hi.astype(np.float32) * dl).astype(bf)
    n2_lod = (n2_lo.astype(np.float32) * dl).astype(bf)
    rowsL += [n2_hi[rowsel], n2_lo[rowsel], ones_i, ones_i]
    rowsR += [ones_j, ones_j, n2_hid, n2_lod]
    return rowsL, rowsR


def _ext_blocks(h):
    return [(ROT[h] + p) % 16 for p in range(NPOS)]


def make_in_maps(q, r):
    """Host-side sharding: per-core augmented bf16 matrices."""
    import ml_dtypes

    bf = ml_dtypes.bfloat16
    q = np.ascontiguousarray(np.asarray(q, np.float32))
    r = np.ascontiguousarray(np.asarray(r, np.float32))
    in_maps = []
    for core in range(NCORES):
        b, h = core // 2, core % 2
        rm = r[b * NB : (b + 1) * NB]
        qm = q[b * NB : (b + 1) * NB]
        hi, lo, n2_hi, n2_lo = _make_mol(rm)

        rowsel = np.concatenate(
            [np.arange(c * 128, (c + 1) * 128) for c in CENTERS[h]]
        )
        colsel = np.concatenate(
            [np.arange(x * 128, (x + 1) * 128) for x in _ext_blocks(h)]
        )
        rowsL, rowsR = _aug_rows(hi, lo, n2_hi, n2_lo, rowsel)
        augs_np = np.ascontiguousarray(np.stack(rowsL).astype(bf))
        augm_np = np.ascontiguousarray(np.stack(rowsR)[:, colsel].astype(bf))

        qi = qm[rowsel]  # [NSLOT*128, NQ] slot-ordered
        q1 = (
            qi.reshape(NSLOT, 128, NQ).transpose(1, 0, 2).reshape(128, NSLOT * NQ)
        ).astype(bf)
        q2 = (2.0 * q1.astype(np.float32)).astype(bf)
        q12_np = np.ascontiguousarray(np.concatenate([q1, q2], axis=1))

        in_maps.append({"augs": augs_np, "augm": augm_np, "q12": q12_np})
    return in_maps


def _host_diag_w(rm):
    """Replicate the device's diagonal weight w'_ii (f32 k-ordered accum)."""
    hi, lo, n2_hi, n2_lo = _make_mol(rm)
    rowsel = np.arange(NB)
    rowsL, rowsR = _aug_rows(hi, lo, n2_hi, n2_lo, rowsel)
    eps = np.zeros(NB, np.float32)
    for L, R in zip(rowsL, rowsR):
        eps = eps + L.astype(np.float32) * R.astype(np.float32)
    eps64 = eps.astype(np.float64)
    p = ((PC0 - eps64) * eps64 + PC1) * eps64 + PC2
    with np.errstate(divide="ignore"):
        y = 1.0 / np.sqrt(np.abs(ACT_SCALE * eps64))
    return np.minimum(y, np.maximum(p, 1.0))


def _unpack_u(raw):
    """uout [72,1024] raw bank dump -> u_ext [NQ, NEXT] (f64)."""
    u = np.zeros((NQ, NEXT), np.float64)
    for band in range(U_BANDS):
        bank, sub = band // 3, band % 3
        w = min(512, NEXT - band * 512)
        u[:, band * 512 : band * 512 + w] = raw[
            32 * sub : 32 * sub + NQ, 512 * bank : 512 * bank + w
        ].astype(np.float64)
    return u


def reduce_outputs(q, r, results):
    """Host-side gather: raw u banks per core -> pot[B]."""
    q = np.asarray(q, np.float32)
    r = np.asarray(r, np.float32)
    TWOPI = 2.0 * np.pi
    pots = np.zeros(B, np.float64)
    for core in range(NCORES):
        b, h = core // 2, core % 2
        u_ext = _unpack_u(results[core]["uout"])
        qm = q[b * NB : (b + 1) * NB].astype(np.float64)
        # fold extended positions back onto blocks
        u_mol = np.zeros((NQ, NB), np.float64)
        for p, x in enumerate(_ext_blocks(h)):
            u_mol[:, x * 128 : (x + 1) * 128] += u_ext[:, p * 128 : (p + 1) * 128]
        pots[b] += (u_mol * qm.T).sum()
    for b in range(B):
        rm = r[b * NB : (b + 1) * NB]
        qm = q[b * NB : (b + 1) * NB].astype(np.float64)
        wdiag = _host_diag_w(rm)
        pots[b] -= ((qm**2).sum(1) * wdiag).sum()
    pots = pots / KAPPA / (4.0 * np.pi)
    for b in range(B):
        qm = q[b * NB : (b + 1) * NB].astype(np.float64)
        pots[b] += (qm**2).sum() / ((2.0 * np.pi) ** 1.5)
    return (pots * NORM_FACTOR).astype(np.float32)


def kernel(q, r, batch):
    global _compiled
    if _compiled is None:
        _compiled = build_nc()
    from concourse import bass_utils

    in_maps = make_in_maps(q, r)
    last_err = None
    for attempt in range(3):
        try:
            res = bass_utils.run_bass_kernel_spmd(
                _compiled, in_maps, core_ids=list(range(NCORES))
            )
            return reduce_outputs(q, r, res.results)
        except Exception as e:  # transient device errors: back off and retry
            last_err = e
            import time

            time.sleep(15 * (attempt + 1))
    raise last_err


# revision 13
# speedup vs baseline: 1.1473x; 1.0857x over previous
"""Trainium2 Bass kernel for the real-space Ewald sum (nn_Ewald) — v2.

Math (per molecule, 2048 atoms, 8 charge channels):
    pot = sum_{i,j} qq_ij * erf(|rij|/sqrt(2)) / (|rij|+1e-6) / (4*pi)
        + sum_i qq_ii / (2*pi)^1.5   (self term), scaled by NORM_FACTOR.

Per pair tile the device computes (as in v1):
    s' = delta*s  (K=13 bf16 hi/lo augmented matmul, delta=2^-3 exact)
    y  = kappa/sqrt(s)        (ACT Abs_reciprocal_sqrt -> bf16)
    w' = min(y, max(P(s'),1)) (one custom DVE op, monic cubic P -> bf16)
    u[c,j] += q[i,c] w'_ij    (PE contraction, f32 PSUM)
Host: pot = (sum(u*q^T) - sum_i qq_ii w'_ii)/kappa/(4*pi) + self term.

v2 geometry — circulant tournament cover at 128-atom granularity:
  16 blocks per molecule; slot centered at block c covers the cyclic
  block window [c, c+8] (9 blocks) for c in 0..7, [c, c+7] (8 blocks)
  for c in 8..15.  Every unordered block pair is covered exactly once
  (difference 1..7 -> lower center; difference 8 -> center < 8; diag ->
  own center with weight 1 via q1, off-center blocks weight 2 via q2).
  Total 68 blocks/core = 8704 pair-columns (vs 10240 in v1, -15%).

SPMD: every core runs slots of widths {9,9,9,9,8,8,8,8} at FIXED window
positions [128j, 128j+1152) (j=0..3) and [128(8+j), 128(8+j)+1024) over
a 19-block extended column space.  Per-core data: augm_ext columns are
the molecule's blocks in cyclic order rotated by 0 (even core: centers
0-3,8-11) or 4 (odd core: centers 4-7,12-15); augs/q12 hold the 8
center blocks.  8 cores = 4 molecules x 2 center sets.

u accumulates in 2 PSUM banks (bands of 512 columns packed at partition
offsets 0/32/64/96 via matmul tile_position); banks are memset to zero
so all matmuls accumulate (start=False).  Host folds the 19-position
extended u back onto the 16 blocks.

PE runs at 1.2 GHz on this platform (HAM clock gate never releases), so
the kernel is PE-stream-bound: A 8704 + B 8704 columns ~= 14.5 us/core.
"""

import numpy as np

B = 4
NB = 2048
NQ = 8
NCORES = 8
NSLOT = 8
NPOS = 19            # extended column space, in 128-blocks
NEXT = NPOS * 128    # 2432 extended columns
NORM_FACTOR = 90.0474
KA = 13              # augmented contraction depth (bf16 hi/lo split)

# fitted scheme constants (see v1): s_c=9, delta=2^-3
DELTA = 0.125
KAPPA = 3.0708577931200534
PC0 = 2.7432632222505378
PC1 = -3.112066562880879
PC2 = 2.4529603188601343
ACT_SCALE = 1.0 / (DELTA * KAPPA * KAPPA)

# slot windows in the extended space: (start, width) in columns
SLOT_WIN = [(128 * j, 1152) for j in range(4)] + [
    (128 * (8 + j), 1024) for j in range(4)
]
# centers per core half h (h=0: rotation 0, h=1: rotation 4)
CENTERS = {0: [0, 1, 2, 3, 8, 9, 10, 11], 1: [4, 5, 6, 7, 12, 13, 14, 15]}
ROT = {0: 0, 1: 4}

U_BANDS = 5          # ceil(2432/512); 3 bands/bank at partition offsets
                     # 0/32/64 (matmul out base partition must be 0/32/64)


def _chunks():
    """Chunk list: (slot t, ext col0, width, w_all offset).

    Main 1024-wide chunks first, the four 128-wide slot tails last: the
    end-of-pipeline drain (A->ACT->DVE->B of the final chunks) is then
    ~4x shorter, and u bank1 (cols >= 1536) closes before the tails.
    """
    mains, tails = [], []
    for t, (ws, w) in enumerate(SLOT_WIN):
        main = min(1024, w)
        mains.append((t, ws, main))
        if w > main:
            tails.append((t, ws + main, w - main))
    out = []
    acc = 0
    for t, c0, cw in mains + tails:
        out.append((t, c0, cw, acc))
        acc += cw
    return out


CHUNKS = _chunks()
NCOLS = sum(c[2] for c in CHUNKS)    # 8704


def _b_pieces(t, col0, cw):
    """B pieces for a chunk: (qsel, piece col0, width); splits at the
    512 band grid and at the 128-wide q1 (diag) prefix of each slot."""
    ws, w = SLOT_WIN[t]
    end = col0 + cw
    pieces = []
    p = col0
    if col0 == ws:                      # first chunk: q1 diag prefix
        pieces.append((1, p, 128))
        p += 128
    while p < end:
        nxt = min(end, (p // 512 + 1) * 512)
        pieces.append((2, p, nxt - p))
        p = nxt
    return pieces


_compiled = None
_ops = None


def _register_ops():
    """Register the fused Ewald-weight DVE op (idempotent)."""
    global _ops
    if _ops is not None:
        return _ops
    from concourse import dve_ops
    from concourse.dve_spec import (
        Spec, Src0, Src1, C0, C1, C2, One, lower, _has_src1, minn, maxx,
    )
    from concourse.dve_uop import DveOpSpec

    def mk(name, spec):
        for o in dve_ops.OPS:
            if o.name == name:
                return o
        shas = {}
        for ver in ("v3", "v4"):
            tmp = DveOpSpec(
                name=name,
                opcode=31,
                uops=lower(spec, ver=ver),
                rd1_en=_has_src1(spec),
            )
            shas[ver] = tmp.sha(ver)
        op = dve_ops.DveOp(name, spec, subdim=False, uops_sha=shas)
        dve_ops.OPS.append(op)
        dve_ops._SUB_OPCODE_FOR_NAME[name] = (
            dve_ops._CUSTOM_DVE_ROW_BASE + len(dve_ops.OPS) - 1
        )
        dve_ops.CUSTOM_DVE_SPECS[name] = spec
        return op

    # w' = min(y, max(((C0 - s')s' + C1)s' + C2, 1))
    def _pw_ref(in0, in1, s0, s1, imm2):
        sp = in0.astype(np.float32)
        y = in1.astype(np.float32)
        p = ((np.float32(s0) - sp) * sp + np.float32(s1)) * sp + np.float32(imm2)
        return np.minimum(y, np.maximum(p, np.float32(1.0))).astype(np.float32)

    ewald_pw = mk(
        "EWALD_PW",
        Spec(
            body=minn(Src1, maxx(((C0 - Src0) * Src0 + C1) * Src0 + C2, One)),
            reference=_pw_ref,
        ),
    )
    _ops = (ewald_pw,)
    return _ops


def build_nc():
    """Build + compile the per-core Bass program (fixed shapes, SPMD)."""
    from concourse import bacc, tile
    import concourse.mybir as mybir
    from concourse.bass import ds
    from concourse.tile_rust import add_dep_helper

    (ewald_pw,) = _register_ops()
    f32 = mybir.dt.float32
    bf16 = mybir.dt.bfloat16
    AF = mybir.ActivationFunctionType

    nc = bacc.Bacc(
        "TRN2",
        target_bir_lowering=False,
        debug=False,
        num_devices=NCORES,
    )
    f16 = mybir.dt.float16
    augs = nc.dram_tensor("augs", [KA, NSLOT * 128], bf16, kind="ExternalInput").ap()
    augm = nc.dram_tensor("augm", [KA, NEXT], bf16, kind="ExternalInput").ap()
    q12 = nc.dram_tensor("q12", [128, 2 * NSLOT * NQ], bf16, kind="ExternalInput").ap()
    # raw dumps of the two u PSUM banks (bands packed at partition offsets)
    uout0 = nc.dram_tensor("uout0", [72, 512], f16, kind="ExternalOutput").ap()
    uout1 = nc.dram_tensor("uout1", [40, 512], f16, kind="ExternalOutput").ap()

    with tile.TileContext(nc) as tc:
        with (
            tc.tile_pool(name="const", bufs=1) as cpool,
            tc.tile_pool(name="ps", bufs=1, space="PSUM") as ps,
        ):
            # ---- input DMAs: augs (A stationary) and augm head first;
            # q12 rides the scalar queue ahead of the ACT table load.
            augs_sb = cpool.tile([KA, NSLOT * 128], bf16)
            nc.gpsimd.dma_start(out=augs_sb[:], in_=augs[:])
            augm_sb = cpool.tile([KA, NEXT], bf16)
            nc.sync.dma_start(out=augm_sb[:, 0:1024], in_=augm[:, 0:1024])
            q12_sb = cpool.tile([128, 2 * NSLOT * NQ], bf16)
            nc.scalar.dma_start(out=q12_sb[:], in_=q12[:])
            nc.sync.dma_start(out=augm_sb[:, 1024:NEXT], in_=augm[:, 1024:NEXT])

            # ---- u banks: bands of 512 at partition offsets, zeroed
            u_banks = [
                ps.tile([128, 512], f32, tag=f"u{i}", bufs=1, name=f"u{i}")
                for i in range(2)
            ]
            nc.vector.memset(u_banks[0][:], 0.0)
            nc.vector.memset(u_banks[1][:], 0.0)

            # flat y/w regions (no rotation -> no reuse deps)
            y_all = cpool.tile([128, NCOLS], bf16)
            w_all = cpool.tile([128, NCOLS], bf16)

            prev_mm = [None]

            def pin(inst):
                if prev_mm[0] is not None:
                    add_dep_helper(
                        inst.ins, prev_mm[0].ins, sync=False,
                        reason="tensor queue order",
                    )
                prev_mm[0] = inst

            n_chunks = len(CHUNKS)
            s_tiles = {}

            def emit_a(k):
                t, col0, cw, _ = CHUNKS[k]
                s_ps = ps.tile([128, 1024], f32, tag="s", bufs=3, name="s_ps")
                s_tiles[k] = s_ps
                off = 0
                while off < cw:
                    pw = min(512, cw - off)
                    pin(nc.tensor.matmul(
                        s_ps[:, ds(off, pw)],
                        augs_sb[:, ds(t * 128, 128)],
                        augm_sb[:, ds(col0 + off, pw)],
                        start=True,
                        stop=True,
                    ))
                    off += pw

            def emit_act_dve(k):
                t, col0, cw, woff = CHUNKS[k]
                s_ps = s_tiles[k]
                nc.scalar.activation(
                    y_all[:, ds(woff, cw)], s_ps[:, 0:cw],
                    AF.Abs_reciprocal_sqrt, scale=ACT_SCALE,
                )
                nc.vector._custom_dve(
                    ewald_pw,
                    out=w_all[:, ds(woff, cw)],
                    in0=s_ps[:, 0:cw],
                    in1=y_all[:, ds(woff, cw)],
                    s0=PC0,
                    s1=PC1,
                    imm2=PC2,
                )

            def emit_b(k):
                t, col0, cw, woff = CHUNKS[k]
                for qsel, pc0, pcw in _b_pieces(t, col0, cw):
                    band = pc0 // 512
                    bank, sub = band // 3, band % 3
                    qoff = (0 if qsel == 1 else NSLOT * NQ) + t * NQ
                    pin(nc.tensor.matmul(
                        u_banks[bank][ds(32 * sub, 8), ds(pc0 - 512 * band, pcw)],
                        q12_sb[:, ds(qoff, NQ)],
                        w_all[:, ds(woff + (pc0 - col0), pcw)],
                        start=False,
                        stop=False,
                        skip_group_check=True,
                    ))
                s_tiles.pop(k)

            # bank1 (cols >= 1536) is only written by the s4-s7 main
            # chunks (indices 4-7); evict it as soon as B(7) is emitted
            # so its copy+DMA overlap the tail chunks.
            staged0 = cpool.tile([128, 512], f16)
            staged1 = cpool.tile([128, 512], f16)

            def emit_b_evict(k):
                emit_b(k)
                if k == 7:
                    nc.scalar.copy(staged1[:], u_banks[1][:])
                    nc.gpsimd.dma_start(out=uout1[:], in_=staged1[0:40, :])

            LAG = 3
            for k in range(n_chunks):
                if k >= LAG:
                    emit_b_evict(k - LAG)
                emit_a(k)
                emit_act_dve(k)
            for k in range(n_chunks - LAG, n_chunks):
                emit_b_evict(k)

            nc.scalar.copy(staged0[:], u_banks[0][:])
            nc.sync.dma_start(out=uout0[:], in_=staged0[0:72, :])

    nc.compile()
    return nc


def _make_mol(rm):
    """Per-molecule hi/lo split (shared by in-map builder and host diag)."""
    import ml_dtypes

    bf = ml_dtypes.bfloat16
    rc = (rm - rm.mean(0, keepdims=True)).astype(np.float32)
    hi = rc.astype(bf)
    lo = (rc - hi.astype(np.float32)).astype(bf)
    rr = hi.astype(np.float32) + lo.astype(np.float32)
    n2 = (rr * rr).sum(1).astype(np.float32)
    n2_hi = n2.astype(bf)
    n2_lo = (n2 - n2_hi.astype(np.float32)).astype(bf)
    return hi, lo, n2_hi, n2_lo


def _aug_rows(hi, lo, n2_hi, n2_lo, rowsel):
    """The 13 (L, R) aug row pairs; R is scaled by DELTA (exact: 2^-3)."""
    import ml_dtypes

    bf = ml_dtypes.bfloat16
    n = hi.shape[0]
    dl = np.float32(DELTA)
    ones_i = np.ones(len(rowsel), bf)
    ones_j = np.full(n, dl, np.float32).astype(bf)
    rowsL, rowsR = [], []
    for ax in range(3):
        m2h = (-2.0 * hi[:, ax].astype(np.float32)).astype(bf)
        m2l = (-2.0 * lo[:, ax].astype(np.float32)).astype(bf)
        m2hd = (m2h.astype(np.float32) * dl).astype(bf)
        m2ld = (m2l.astype(np.float32) * dl).astype(bf)
        rowsL += [hi[rowsel, ax], hi[rowsel, ax], lo[rowsel, ax]]
        rowsR += [m2hd, m2ld, m2hd]
    n2_hid = (n2_hi.astype(np.float32) * dl).astype(bf)
    n2_lod = (n2_lo.astype(np.float32) * dl).astype(bf)
    rowsL += [n2_hi[rowsel], n2_lo[rowsel], ones_i, ones_i]
    rowsR += [ones_j, ones_j, n2_hid, n2_lod]
    return rowsL, rowsR


def _ext_blocks(h):
    return [(ROT[h] + p) % 16 for p in range(NPOS)]


def make_in_maps(q, r):
    """Host-side sharding: per-core augmented bf16 matrices."""
    import ml_dtypes

    bf = ml_dtypes.bfloat16
    q = np.ascontiguousarray(np.asarray(q, np.float32))
    r = np.ascontiguousarray(np.asarray(r, np.float32))
    in_maps = []
    for core in range(NCORES):
        b, h = core // 2, core % 2
        rm = r[b * NB : (b + 1) * NB]
        qm = q[b * NB : (b + 1) * NB]
        hi, lo, n2_hi, n2_lo = _make_mol(rm)

        rowsel = np.concatenate(
            [np.arange(c * 128, (c + 1) * 128) for c in CENTERS[h]]
        )
        colsel = np.concatenate(
            [np.arange(x * 128, (x + 1) * 128) for x in _ext_blocks(h)]
        )
        rowsL, rowsR = _aug_rows(hi, lo, n2_hi, n2_lo, rowsel)
        augs_np = np.ascontiguousarray(np.stack(rowsL).astype(bf))
        augm_np = np.ascontiguousarray(np.stack(rowsR)[:, colsel].astype(bf))

        qi = qm[rowsel]  # [NSLOT*128, NQ] slot-ordered
        q1 = (
            qi.reshape(NSLOT, 128, NQ).transpose(1, 0, 2).reshape(128, NSLOT * NQ)
        ).astype(bf)
        q2 = (2.0 * q1.astype(np.float32)).astype(bf)
        q12_np = np.ascontiguousarray(np.concatenate([q1, q2], axis=1))

        in_maps.append({"augs": augs_np, "augm": augm_np, "q12": q12_np})
    return in_maps


def _host_diag_w(rm):
    """Replicate the device's diagonal weight w'_ii (f32 k-ordered accum)."""
    hi, lo, n2_hi, n2_lo = _make_mol(rm)
    rowsel = np.arange(NB)
    rowsL, rowsR = _aug_rows(hi, lo, n2_hi, n2_lo, rowsel)
    eps = np.zeros(NB, np.float32)
    for L, R in zip(rowsL, rowsR):
        eps = eps + L.astype(np.float32) * R.astype(np.float32)
    eps64 = eps.astype(np.float64)
    p = ((PC0 - eps64) * eps64 + PC1) * eps64 + PC2
    with np.errstate(divide="ignore"):
        y = 1.0 / np.sqrt(np.abs(ACT_SCALE * eps64))
    return np.minimum(y, np.maximum(p, 1.0))


def _unpack_u(raw0, raw1):
    """uout0 [72,512] + uout1 [40,512] bank dumps -> u_ext [NQ, NEXT]."""
    u = np.zeros((NQ, NEXT), np.float64)
    for band in range(U_BANDS):
        bank, sub = band // 3, band % 3
        w = min(512, NEXT - band * 512)
        raw = raw0 if bank == 0 else raw1
        u[:, band * 512 : band * 512 + w] = raw[
            32 * sub : 32 * sub + NQ, 0:w
        ].astype(np.float64)
    return u


def reduce_outputs(q, r, results):
    """Host-side gather: raw u banks per core -> pot[B]."""
    q = np.asarray(q, np.float32)
    r = np.asarray(r, np.float32)
    TWOPI = 2.0 * np.pi
    pots = np.zeros(B, np.float64)
    for core in range(NCORES):
        b, h = core // 2, core % 2
        u_ext = _unpack_u(results[core]["uout0"], results[core]["uout1"])
        qm = q[b * NB : (b + 1) * NB].astype(np.float64)
        # fold extended positions back onto blocks
        u_mol = np.zeros((NQ, NB), np.float64)
        for p, x in enumerate(_ext_blocks(h)):
            u_mol[:, x * 128 : (x + 1) * 128] += u_ext[:, p * 128 : (p + 1) * 128]
        pots[b] += (u_mol * qm.T).sum()
    for b in range(B):
        rm = r[b * NB : (b + 1) * NB]
        qm = q[b * NB : (b + 1) * NB].astype(np.float64)
        wdiag = _host_diag_w(rm)
        pots[b] -= ((qm**2).sum(1) * wdiag).sum()
    pots = pots / KAPPA / (4.0 * np.pi)
    for b in range(B):
        qm = q[b * NB : (b + 1) * NB].astype(np.float64)
        pots[b] += (qm**2).sum() / ((2.0 * np.pi) ** 1.5)
    return (pots * NORM_FACTOR).astype(np.float32)


def kernel(q, r, batch):
    global _compiled
    if _compiled is None:
        _compiled = build_nc()
    from concourse import bass_utils

    in_maps = make_in_maps(q, r)
    last_err = None
    for attempt in range(3):
        try:
            res = bass_utils.run_bass_kernel_spmd(
                _compiled, in_maps, core_ids=list(range(NCORES))
            )
            return reduce_outputs(q, r, res.results)
        except Exception as e:  # transient device errors: back off and retry
            last_err = e
            import time

            time.sleep(15 * (attempt + 1))
    raise last_err
